# revision 38
# baseline (speedup 1.0000x reference)
"""Multi-head causal attention on 8 trn2 NeuronCores.

Reference semantics (B=2, S=2048, D=1024, H=16, DK=DV=64):
    q = X @ WQ * 1/sqrt(DK); k = X @ WK; v = X @ WV          (per head)
    logits[i, j] = q[i] . k[j]   (i = key pos, j = query pos, causal i <= j)
    P = softmax_i(logits); out[j] = (sum_i P[i,j] v[i]) @ WO + bO

Sharding: 2 batches x 16 heads = 32 bh-pairs -> 4 heads/core, batch b = core//4.
Each core computes attention for its heads plus the partial output projection
x_part @ WO[rows of its heads]; the host sums the 4 partials per batch
(all-reduce step of the row-sharded WO) and adds bO.

Device layout per core:
    XT  [D, S]       input transposed (d on partitions)
    QT/KT [hd=256, S] head-major transposed projections (d_head on partitions)
    V   [S, 4, 65]   natural layout + ones column (col 64) -> matmul row-sums
    scores^T psum [i=128, 2 heads, 512 j]  -> exp on ScalarE -> PT bf16
    x~ psum [j=128, 4 heads, 65]: accumulate PT.T @ V_aug over i-chunks;
       col 64 = softmax denominators -> reciprocal -> scale -> x_n bf16
    x_n -> PE transpose -> xT [256, S] -> out = xT.T @ WO_part -> DRAM f32
"""

import functools

import numpy as np
import ml_dtypes

import concourse.bass as bass
import concourse.mybir as mybir
import concourse.tile as tile
from concourse import bacc
from concourse.bass_utils import run_bass_kernel_spmd
from concourse.masks import make_identity

B, S, D, H = 2, 2048, 1024, 16
DK = DV = 64
NCORES = 8
GROUP = NCORES // B          # cores per batch
HG = H // GROUP              # heads per core = 4
HD = HG * DK                 # per-core head dims = 256
P = 128
KC = D // P                  # 8 contraction chunks over D
JB = 512                     # query-block width for score matmuls
NJB = S // JB                # 4
NIC = S // P                 # 16 key chunks
NJC = S // P                 # 16 query chunks
VW = DV + 1                  # value width + ones column

BF16 = mybir.dt.bfloat16
F32 = mybir.dt.float32
NPBF16 = ml_dtypes.bfloat16
Exp = mybir.ActivationFunctionType.Exp


def build_nc() -> bass.Bass:
    nc = bacc.Bacc()
    xt = nc.declare_dram_parameter("xt", [D, S], BF16, isOutput=False)
    wq = nc.declare_dram_parameter("wq", [D, HD], BF16, isOutput=False)
    wk = nc.declare_dram_parameter("wk", [D, HD], BF16, isOutput=False)
    wv = nc.declare_dram_parameter("wv", [D, HD], BF16, isOutput=False)
    wo = nc.declare_dram_parameter("wo", [HD, D], BF16, isOutput=False)
    tri = nc.declare_dram_parameter("tri", [P, P], BF16, isOutput=False)
    out = nc.declare_dram_parameter("out_part", [S, D], F32, isOutput=True)

    out_t = out.rearrange("(c p) o -> p c o", p=P)

    with tile.TileContext(nc) as tc:
        with (
            tc.tile_pool(name="const", bufs=1) as const_pool,
            tc.tile_pool(name="big", bufs=1) as big_pool,
            tc.tile_pool(name="pt", bufs=40) as pt_pool,
            tc.tile_pool(name="small", bufs=8) as small_pool,
            tc.tile_pool(name="osb", bufs=4) as osb_pool,
            tc.tile_pool(name="mmps", bufs=2, space="PSUM") as mm_psum,
            tc.tile_pool(name="sps", bufs=2, space="PSUM") as s_psum,
            tc.tile_pool(name="avps", bufs=2, space="PSUM") as av_psum,
        ):
            ident = const_pool.tile([P, P], BF16)
            make_identity(nc, ident)
            tri_sb = const_pool.tile([P, P], BF16)
            nc.sync.dma_start(tri_sb, tri[:, :])

            xt_t = xt.rearrange("(kc p) i -> p kc i", p=P)
            w_sbs = {
                name: big_pool.tile([P, KC, HD], BF16, name=f"{name}_sb")
                for name in ("wq", "wk", "wv")
            }
            xt_sbs = [
                big_pool.tile([P, KC, JB], BF16, name=f"xt_sb{nb}")
                for nb in range(NJB)
            ]
            wo_sb = big_pool.tile([P, HD // P, D], BF16, name="wo_sb")
            wk_t = wk.rearrange("(kc p) m -> p kc m", p=P)
            nc.sync.dma_start(w_sbs["wk"][:, :, 0:P], wk_t[:, :, 0:P])
            nc.sync.dma_start(xt_sbs[0][:, 0 : KC // 2, :], xt_t[:, 0 : KC // 2, 0:JB])
            nc.sync.dma_start(xt_sbs[0][:, KC // 2 :, :], xt_t[:, KC // 2 :, 0:JB])
            nc.sync.dma_start(w_sbs["wk"][:, :, P:HD], wk_t[:, :, P:HD])
            nc.sync.dma_start(w_sbs["wq"], wq.rearrange("(kc p) m -> p kc m", p=P))
            nc.sync.dma_start(w_sbs["wv"], wv.rearrange("(kc p) m -> p kc m", p=P))
            for nb in range(1, NJB):
                nc.sync.dma_start(xt_sbs[nb], xt_t[:, :, nb * JB : (nb + 1) * JB])
            nc.sync.dma_start(wo_sb, wo.rearrange("(hc p) o -> p hc o", p=P))

            qt_sb = big_pool.tile([P, HD // P, S], BF16, name="qt_sb")
            kt_sb = big_pool.tile([P, HD // P, S], BF16, name="kt_sb")
            v_sb = big_pool.tile([P, NIC, HG, VW], BF16, name="v_sb")
            nc.vector.memset(v_sb[:, :, :, DV : DV + 1], 1.0)
            xT_sb = big_pool.tile([P, HD // P, S], BF16, name="xT_sb")

            def qtkt_chain(nb, w_sb, t_sb, mc):
                ps = mm_psum.tile([P, JB], F32, tag="mmps")
                for kc in range(KC):
                    nc.tensor.matmul(
                        ps,
                        lhsT=w_sb[:, kc, mc * P : (mc + 1) * P],
                        rhs=xt_sbs[nb][:, kc, :],
                        start=(kc == 0),
                        stop=(kc == KC - 1),
                    )
                nc.vector.tensor_copy(t_sb[:, mc, nb * JB : (nb + 1) * JB], ps)

            def v_chain(ic):
                ps = mm_psum.tile([P, JB], F32, tag="mmps")
                for kc in range(KC):
                    nc.tensor.matmul(
                        ps[:, :HD],
                        lhsT=xt_sbs[ic // 4][:, kc, (ic % 4) * P : (ic % 4 + 1) * P],
                        rhs=w_sbs["wv"][:, kc, :],
                        start=(kc == 0),
                        stop=(kc == KC - 1),
                    )
                nc.vector.tensor_copy(
                    v_sb[:, ic, :, 0:DV],
                    ps[:, :HD].rearrange("p (h v) -> p h v", v=DV),
                )

            def proj_chains(nb):
                """KT first (needed by every score of block nb), then QT, V."""
                yield lambda: qtkt_chain(nb, w_sbs["wk"], kt_sb, 0)
                yield lambda: qtkt_chain(nb, w_sbs["wk"], kt_sb, 1)
                yield lambda: qtkt_chain(nb, w_sbs["wq"], qt_sb, 0)
                yield lambda: qtkt_chain(nb, w_sbs["wq"], qt_sb, 1)
                for ic in range(4 * nb, 4 * nb + 4):
                    yield lambda ic=ic: v_chain(ic)

            for c in proj_chains(0):
                c()

            pt_tiles = {}  # (jb, ib, hp) -> exp'd probability tile

            def score_tile(jb, ib):
                """Scores + exp for one (query block, key chunk), both head
                pairs. Trimmed to the causally live j-range."""
                off = max(0, (ib - 4 * jb) * P)
                for hp in range(HG // 2):  # pack 2 heads per psum tile
                    sps = s_psum.tile([P, 2, JB], F32, tag="sps")
                    for hh in range(2):
                        h = 2 * hp + hh
                        base = DK * (h % 2)
                        hc = h // 2
                        nc.tensor.matmul(
                            sps[:, hh, off:],
                            lhsT=qt_sb[base : base + DK, hc, ib * P : (ib + 1) * P],
                            rhs=kt_sb[base : base + DK, hc, jb * JB + off : (jb + 1) * JB],
                            start=True,
                            stop=True,
                        )
                    pt = pt_pool.tile([P, 2, JB], BF16, tag="pt")
                    nc.scalar.activation(pt[:, :, off:], sps[:, :, off:], Exp)
                    pt_tiles[jb, ib, hp] = pt

            # --- attention, one 512-wide query block at a time. Fillers keep
            # PE (projection chains) and ScalarE (next block's first score
            # tiles, emitted during the AV section) busy across boundaries ---
            PREFETCH = 7
            for jb in range(NJB):
                nib = 4 * jb + 4  # causal: key chunks 0 .. 4*jb+3
                filler = proj_chains(jb + 1) if jb + 1 < NJB else iter(())
                for ib in range(nib):
                    if (jb, ib, 0) not in pt_tiles:
                        score_tile(jb, ib)
                    c = next(filler, None)
                    if c is not None:
                        c()
                for c in filler:  # leftovers (jb=0 has fewer score slots)
                    c()

                # last block: do the longest chunk (jc=15, which depends on
                # the final score tile) first so the kernel doesn't end on it
                jjs = (3, 0, 1, 2) if jb == NJB - 1 else range(4)
                for jj in jjs:
                    if jb + 1 < NJB and jj < PREFETCH:
                        score_tile(jb + 1, jj)
                    jc = 4 * jb + jj
                    comb = av_psum.tile([P, 448], F32, tag="avps")
                    xps = comb[:, 0 : HG * VW].rearrange("p (h v) -> p h v", v=VW)
                    for h in range(HG):
                        hp, hh = divmod(h, 2)
                        for ib in range(jc + 1):
                            lhsT = pt_tiles[jb, ib, hp][:, hh, jj * P : (jj + 1) * P]
                            if ib == jc:  # diagonal: zero the i > j half
                                ptd = small_pool.tile([P, P], BF16, tag="ptd")
                                nc.vector.tensor_mul(ptd, lhsT, tri_sb)
                                lhsT = ptd
                            nc.tensor.matmul(
                                xps[:, h, :],
                                lhsT=lhsT,
                                rhs=v_sb[:, ib, h, :],
                                start=(ib == 0),
                                stop=(ib == jc),
                            )
                    recip = small_pool.tile([P, HG], F32, tag="recip")
                    nc.vector.reciprocal(recip, xps[:, :, DV])
                    xn = small_pool.tile([P, HG, DV], BF16, tag="xn")
                    nc.vector.tensor_tensor(
                        xn,
                        xps[:, :, 0:DV],
                        recip[:, :, None].to_broadcast([P, HG, DV]),
                        mybir.AluOpType.mult,
                    )
                    xn_flat = xn.rearrange("p h v -> p (h v)")
                    for vc in range(HD // P):
                        tps = comb[:, 288 + vc * DV : 288 + (vc + 1) * DV].bitcast(BF16)
                        nc.tensor.transpose(tps, xn_flat[:, vc * P : (vc + 1) * P], ident)
                        nc.vector.tensor_copy(xT_sb[:, vc, jc * P : (jc + 1) * P], tps)

                    # partial output projection for this query chunk
                    for oc in range(D // JB):
                        ops = mm_psum.tile([P, JB], F32, tag="mmps")
                        for hc in range(HD // P):
                            nc.tensor.matmul(
                                ops,
                                lhsT=xT_sb[:, hc, jc * P : (jc + 1) * P],
                                rhs=wo_sb[:, hc, oc * JB : (oc + 1) * JB],
                                start=(hc == 0),
                                stop=(hc == HD // P - 1),
                            )
                        osb = osb_pool.tile([P, JB], F32, tag="osb")
                        nc.vector.tensor_copy(osb, ops)
                        nc.sync.dma_start(out_t[:, jc, oc * JB : (oc + 1) * JB], osb)
    nc.compile()
    return nc


@functools.lru_cache(maxsize=1)
def _cached_nc() -> bass.Bass:
    return build_nc()


def make_in_maps(inputs, mask, WQ, WK, WV, WO, bO):
    scale = np.float32(1.0 / np.sqrt(DK))
    wq2 = np.ascontiguousarray((WQ.reshape(D, D) * scale).astype(NPBF16))
    wk2 = np.ascontiguousarray(WK.reshape(D, D).astype(NPBF16))
    wv2 = np.ascontiguousarray(WV.reshape(D, D).astype(NPBF16))
    wo2 = np.ascontiguousarray(WO.astype(NPBF16))
    tri = np.triu(np.ones((P, P), np.float32)).astype(NPBF16)
    xts = [
        np.ascontiguousarray(np.asarray(inputs[b]).T.astype(NPBF16)) for b in range(B)
    ]
    in_maps = []
    for c in range(NCORES):
        b, hg = divmod(c, GROUP)
        cols = slice(hg * HD, (hg + 1) * HD)
        in_maps.append(
            {
                "xt": xts[b],
                "wq": np.ascontiguousarray(wq2[:, cols]),
                "wk": np.ascontiguousarray(wk2[:, cols]),
                "wv": np.ascontiguousarray(wv2[:, cols]),
                "wo": np.ascontiguousarray(wo2[cols, :]),
                "tri": tri,
            }
        )
    return in_maps


def combine(results, bO):
    parts = [r["out_part"] for r in results]
    out = np.empty((B, S, D), np.float32)
    for b in range(B):
        acc = parts[b * GROUP].astype(np.float32).copy()
        for g in range(1, GROUP):
            acc += parts[b * GROUP + g]
        out[b] = acc + np.asarray(bO, np.float32)[None, :]
    return out


def kernel(**inputs) -> np.ndarray:
    nc = _cached_nc()
    in_maps = make_in_maps(**inputs)
    res = run_bass_kernel_spmd(nc, in_maps, core_ids=list(range(NCORES)))
    return combine(res.results, inputs["bO"])


# revision 42
# speedup vs baseline: 1.0053x; 1.0053x over previous
"""Multi-head causal attention on 8 trn2 NeuronCores.

Reference semantics (B=2, S=2048, D=1024, H=16, DK=DV=64):
    q = X @ WQ * 1/sqrt(DK); k = X @ WK; v = X @ WV          (per head)
    logits[i, j] = q[i] . k[j]   (i = key pos, j = query pos, causal i <= j)
    P = softmax_i(logits); out[j] = (sum_i P[i,j] v[i]) @ WO + bO

Sharding: 2 batches x 16 heads = 32 bh-pairs -> 4 heads/core, batch b = core//4.
Each core computes attention for its heads plus the partial output projection
x_part @ WO[rows of its heads]; the host sums the 4 partials per batch
(all-reduce step of the row-sharded WO) and adds bO.

Device layout per core:
    XT  [D, S]       input transposed (d on partitions)
    QT/KT [hd=256, S] head-major transposed projections (d_head on partitions)
    V   [S, 4, 65]   natural layout + ones column (col 64) -> matmul row-sums
    scores^T psum [i=128, 2 heads, 512 j]  -> exp on ScalarE -> PT bf16
    x~ psum [j=128, 4 heads, 65]: accumulate PT.T @ V_aug over i-chunks;
       col 64 = softmax denominators -> reciprocal -> scale -> x_n bf16
    x_n -> PE transpose -> xT [256, S] -> out = xT.T @ WO_part -> DRAM f32
"""

import functools

import numpy as np
import ml_dtypes

import concourse.bass as bass
import concourse.mybir as mybir
import concourse.tile as tile
from concourse import bacc
from concourse.bass_utils import run_bass_kernel_spmd
from concourse.masks import make_identity

B, S, D, H = 2, 2048, 1024, 16
DK = DV = 64
NCORES = 8
GROUP = NCORES // B          # cores per batch
HG = H // GROUP              # heads per core = 4
HD = HG * DK                 # per-core head dims = 256
P = 128
KC = D // P                  # 8 contraction chunks over D
JB = 512                     # query-block width for score matmuls
NJB = S // JB                # 4
NIC = S // P                 # 16 key chunks
NJC = S // P                 # 16 query chunks
VW = DV + 1                  # value width + ones column

BF16 = mybir.dt.bfloat16
F32 = mybir.dt.float32
NPBF16 = ml_dtypes.bfloat16
Exp = mybir.ActivationFunctionType.Exp


def build_nc() -> bass.Bass:
    nc = bacc.Bacc()
    xt = nc.declare_dram_parameter("xt", [D, S], BF16, isOutput=False)
    wq = nc.declare_dram_parameter("wq", [D, HD], BF16, isOutput=False)
    wk = nc.declare_dram_parameter("wk", [D, HD], BF16, isOutput=False)
    wv = nc.declare_dram_parameter("wv", [D, HD], BF16, isOutput=False)
    wo = nc.declare_dram_parameter("wo", [HD, D], BF16, isOutput=False)
    tri = nc.declare_dram_parameter("tri", [P, P], BF16, isOutput=False)
    out = nc.declare_dram_parameter("out_part", [S, D], F32, isOutput=True)

    out_t = out.rearrange("(c p) o -> p c o", p=P)

    with tile.TileContext(nc) as tc:
        with (
            tc.tile_pool(name="const", bufs=1) as const_pool,
            tc.tile_pool(name="big", bufs=1) as big_pool,
            tc.tile_pool(name="pt", bufs=42) as pt_pool,
            tc.tile_pool(name="small", bufs=8) as small_pool,
            tc.tile_pool(name="osb", bufs=6) as osb_pool,
            tc.tile_pool(name="mmps", bufs=2, space="PSUM") as mm_psum,
            tc.tile_pool(name="sps", bufs=2, space="PSUM") as s_psum,
            tc.tile_pool(name="avps", bufs=2, space="PSUM") as av_psum,
        ):
            ident = const_pool.tile([P, P], BF16)
            make_identity(nc, ident)
            tri_sb = const_pool.tile([P, P], BF16)
            nc.sync.dma_start(tri_sb, tri[:, :])

            xt_t = xt.rearrange("(kc p) i -> p kc i", p=P)
            w_sbs = {
                name: big_pool.tile([P, KC, HD], BF16, name=f"{name}_sb")
                for name in ("wq", "wk", "wv")
            }
            xt_sbs = [
                big_pool.tile([P, KC, JB], BF16, name=f"xt_sb{nb}")
                for nb in range(NJB)
            ]
            wo_sb = big_pool.tile([P, HD // P, D], BF16, name="wo_sb")
            wk_t = wk.rearrange("(kc p) m -> p kc m", p=P)
            nc.sync.dma_start(w_sbs["wk"][:, :, 0:P], wk_t[:, :, 0:P])
            nc.sync.dma_start(xt_sbs[0][:, 0 : KC // 2, :], xt_t[:, 0 : KC // 2, 0:JB])
            nc.sync.dma_start(xt_sbs[0][:, KC // 2 :, :], xt_t[:, KC // 2 :, 0:JB])
            nc.sync.dma_start(w_sbs["wk"][:, :, P:HD], wk_t[:, :, P:HD])
            nc.sync.dma_start(w_sbs["wq"], wq.rearrange("(kc p) m -> p kc m", p=P))
            nc.sync.dma_start(w_sbs["wv"], wv.rearrange("(kc p) m -> p kc m", p=P))
            for nb in range(1, NJB):
                nc.sync.dma_start(xt_sbs[nb], xt_t[:, :, nb * JB : (nb + 1) * JB])
            nc.sync.dma_start(wo_sb, wo.rearrange("(hc p) o -> p hc o", p=P))

            qt_sb = big_pool.tile([P, HD // P, S], BF16, name="qt_sb")
            kt_sb = big_pool.tile([P, HD // P, S], BF16, name="kt_sb")
            v_sb = big_pool.tile([P, NIC, HG, VW], BF16, name="v_sb")
            nc.vector.memset(v_sb[:, :, :, DV : DV + 1], 1.0)
            xT_sb = big_pool.tile([P, HD // P, S], BF16, name="xT_sb")

            def qtkt_chain(nb, w_sb, t_sb, mc):
                ps = mm_psum.tile([P, JB], F32, tag="mmps")
                for kc in range(KC):
                    nc.tensor.matmul(
                        ps,
                        lhsT=w_sb[:, kc, mc * P : (mc + 1) * P],
                        rhs=xt_sbs[nb][:, kc, :],
                        start=(kc == 0),
                        stop=(kc == KC - 1),
                    )
                nc.vector.tensor_copy(t_sb[:, mc, nb * JB : (nb + 1) * JB], ps)

            def v_chain(ic):
                ps = mm_psum.tile([P, JB], F32, tag="mmps")
                for kc in range(KC):
                    nc.tensor.matmul(
                        ps[:, :HD],
                        lhsT=xt_sbs[ic // 4][:, kc, (ic % 4) * P : (ic % 4 + 1) * P],
                        rhs=w_sbs["wv"][:, kc, :],
                        start=(kc == 0),
                        stop=(kc == KC - 1),
                    )
                nc.vector.tensor_copy(
                    v_sb[:, ic, :, 0:DV],
                    ps[:, :HD].rearrange("p (h v) -> p h v", v=DV),
                )

            def proj_chains(nb):
                """KT first (needed by every score of block nb), then QT, V."""
                yield lambda: qtkt_chain(nb, w_sbs["wk"], kt_sb, 0)
                yield lambda: qtkt_chain(nb, w_sbs["wk"], kt_sb, 1)
                yield lambda: qtkt_chain(nb, w_sbs["wq"], qt_sb, 0)
                yield lambda: qtkt_chain(nb, w_sbs["wq"], qt_sb, 1)
                for ic in range(4 * nb, 4 * nb + 4):
                    yield lambda ic=ic: v_chain(ic)

            for c in proj_chains(0):
                c()

            pt_tiles = {}  # (jb, ib, hp) -> exp'd probability tile

            def score_tile(jb, ib):
                """Scores + exp for one (query block, key chunk), both head
                pairs. Trimmed to the causally live j-range."""
                off = max(0, (ib - 4 * jb) * P)
                for hp in range(HG // 2):  # pack 2 heads per psum tile
                    sps = s_psum.tile([P, 2, JB], F32, tag="sps")
                    for hh in range(2):
                        h = 2 * hp + hh
                        base = DK * (h % 2)
                        hc = h // 2
                        nc.tensor.matmul(
                            sps[:, hh, off:],
                            lhsT=qt_sb[base : base + DK, hc, ib * P : (ib + 1) * P],
                            rhs=kt_sb[base : base + DK, hc, jb * JB + off : (jb + 1) * JB],
                            start=True,
                            stop=True,
                        )
                    pt = pt_pool.tile([P, 2, JB], BF16, tag="pt")
                    nc.scalar.activation(pt[:, :, off:], sps[:, :, off:], Exp)
                    pt_tiles[jb, ib, hp] = pt

            # --- attention, one 512-wide query block at a time. Fillers keep
            # PE (projection chains) and ScalarE (next block's first score
            # tiles, emitted during the AV section) busy across boundaries ---
            PREFETCH = 7
            for jb in range(NJB):
                nib = 4 * jb + 4  # causal: key chunks 0 .. 4*jb+3
                filler = proj_chains(jb + 1) if jb + 1 < NJB else iter(())
                for ib in range(nib):
                    if (jb, ib, 0) not in pt_tiles:
                        score_tile(jb, ib)
                    c = next(filler, None)
                    if c is not None:
                        c()
                for c in filler:  # leftovers (jb=0 has fewer score slots)
                    c()

                for jj in range(4):
                    if jb + 1 < NJB and jj < PREFETCH:
                        score_tile(jb + 1, jj)
                    jc = 4 * jb + jj
                    comb = av_psum.tile([P, 448], F32, tag="avps")
                    xps = comb[:, 0 : HG * VW].rearrange("p (h v) -> p h v", v=VW)
                    for h in range(HG):
                        hp, hh = divmod(h, 2)
                        for ib in range(jc + 1):
                            lhsT = pt_tiles[jb, ib, hp][:, hh, jj * P : (jj + 1) * P]
                            if ib == jc:  # diagonal: zero the i > j half
                                ptd = small_pool.tile([P, P], BF16, tag="ptd")
                                nc.vector.tensor_mul(ptd, lhsT, tri_sb)
                                lhsT = ptd
                            nc.tensor.matmul(
                                xps[:, h, :],
                                lhsT=lhsT,
                                rhs=v_sb[:, ib, h, :],
                                start=(ib == 0),
                                stop=(ib == jc),
                            )
                    recip = small_pool.tile([P, HG], F32, tag="recip")
                    nc.vector.reciprocal(recip, xps[:, :, DV])
                    xn = small_pool.tile([P, HG, DV], BF16, tag="xn")
                    nc.vector.tensor_tensor(
                        xn,
                        xps[:, :, 0:DV],
                        recip[:, :, None].to_broadcast([P, HG, DV]),
                        mybir.AluOpType.mult,
                    )
                    xn_flat = xn.rearrange("p h v -> p (h v)")
                    for vc in range(HD // P):
                        tps = comb[:, 288 + vc * DV : 288 + (vc + 1) * DV].bitcast(BF16)
                        nc.tensor.transpose(tps, xn_flat[:, vc * P : (vc + 1) * P], ident)
                        nc.vector.tensor_copy(xT_sb[:, vc, jc * P : (jc + 1) * P], tps)

                    # partial output projection for this query chunk
                    for oc in range(D // JB):
                        ops = mm_psum.tile([P, JB], F32, tag="mmps")
                        for hc in range(HD // P):
                            nc.tensor.matmul(
                                ops,
                                lhsT=xT_sb[:, hc, jc * P : (jc + 1) * P],
                                rhs=wo_sb[:, hc, oc * JB : (oc + 1) * JB],
                                start=(hc == 0),
                                stop=(hc == HD // P - 1),
                            )
                        osb = osb_pool.tile([P, JB], F32, tag="osb")
                        nc.vector.tensor_copy(osb, ops)
                        nc.sync.dma_start(out_t[:, jc, oc * JB : (oc + 1) * JB], osb)
    nc.compile()
    return nc


@functools.lru_cache(maxsize=1)
def _cached_nc() -> bass.Bass:
    return build_nc()


def make_in_maps(inputs, mask, WQ, WK, WV, WO, bO):
    scale = np.float32(1.0 / np.sqrt(DK))
    wq2 = np.ascontiguousarray((WQ.reshape(D, D) * scale).astype(NPBF16))
    wk2 = np.ascontiguousarray(WK.reshape(D, D).astype(NPBF16))
    wv2 = np.ascontiguousarray(WV.reshape(D, D).astype(NPBF16))
    wo2 = np.ascontiguousarray(WO.astype(NPBF16))
    tri = np.triu(np.ones((P, P), np.float32)).astype(NPBF16)
    xts = [
        np.ascontiguousarray(np.asarray(inputs[b]).T.astype(NPBF16)) for b in range(B)
    ]
    in_maps = []
    for c in range(NCORES):
        b, hg = divmod(c, GROUP)
        cols = slice(hg * HD, (hg + 1) * HD)
        in_maps.append(
            {
                "xt": xts[b],
                "wq": np.ascontiguousarray(wq2[:, cols]),
                "wk": np.ascontiguousarray(wk2[:, cols]),
                "wv": np.ascontiguousarray(wv2[:, cols]),
                "wo": np.ascontiguousarray(wo2[cols, :]),
                "tri": tri,
            }
        )
    return in_maps


def combine(results, bO):
    parts = [r["out_part"] for r in results]
    out = np.empty((B, S, D), np.float32)
    for b in range(B):
        acc = parts[b * GROUP].astype(np.float32).copy()
        for g in range(1, GROUP):
            acc += parts[b * GROUP + g]
        out[b] = acc + np.asarray(bO, np.float32)[None, :]
    return out


def kernel(**inputs) -> np.ndarray:
    nc = _cached_nc()
    in_maps = make_in_maps(**inputs)
    res = run_bass_kernel_spmd(nc, in_maps, core_ids=list(range(NCORES)))
    return combine(res.results, inputs["bO"])
